# revision 20
# baseline (speedup 1.0000x reference)
"""Multi-head attention (batched, key-padding mask) Trainium2 Bass kernel.

Problem: nn_MultiHeadBatched
  q,k,v: [B=4, S=2048, E=1024] fp32; mask: [B, 2048] int32 (key padding)
  16 heads, head_dim 64; torch-Linear style q/k/v/out projections.

Sharding (8 cores): core c handles batch b=c//2 and head group hg=c%2
(8 heads each).  q/k/v projections are column-parallel over the head
group; out-projection is row-parallel — each core produces a partial
[E, Sq] output and the host sums the two partials per batch (+ bo).

v3 structure (single NeuronCore program, SPMD over 8 cores):
  - Host compacts KV per batch to the valid (mask!=0) positions, padded
    to a multiple of 128 (SKV); pad positions get an additive -1e30 exp
    bias (folded into the ScalarE activation).
  - Scores transposed ([kv, q]); softmax denominator Z from an all-ones
    65th column on each head's V (row 64 of the AV accumulation).
  - Head-slot pipeline with HALF-phases: slot h runs
      AV(h-1, strips 0-1) ; scores+exp(h, q-half 0) ;
      AV(h-1, strips 2-3) ; scores+exp(h, q-half 1)
    so the single-buffered P tiles ([128,2048] per kv chunk) free in
    halves just before exp needs them, and ScalarE stays busy across the
    slot boundary (previous half's exps overlap this slot's AV).
  - Q/K projections for head-pairs 1-3 are background items woven into
    the j-loops, so the exp stream starts ~15us into the kernel.
  - AV runs j-major per q-half into one [128,1024] PSUM tile (2 banks);
    V is augmented per head to [64 V | 64 ones] columns so the AV output
    carries Z replicated across rows 64-127, partition-aligned with A.
    Each half is normalized immediately: Z rows -> base-0 SBUF (standard
    cross-partition copy), custom-DVE reciprocal_approx_fast (base-0
    operands only — ISA lowering drops base_partition), then one DVE mul
    reading A straight from PSUM into aall (bf16).
  - PSUM: scores 2x[128,1024] (4) + AV [128,1024] (2) + proj 2x[128,512]
    (2) = 8 banks.
"""

import os
import sys

import numpy as np

sys.path.insert(0, "/opt/trn_rl_repo")

import concourse.bass as bass
import concourse.bacc as bacc
import concourse.mybir as mybir
import concourse.tile as tile
from concourse import bass_utils

import ml_dtypes

BF16 = ml_dtypes.bfloat16

B, SQ, E = 4, 2048, 1024
H_TOT, D = 16, 64
HPC = H_TOT // 2            # heads per core (head-group split in 2)
DHC = HPC * D               # 512 projected channels per core
NE = E // 128               # contraction chunks
NDH = DHC // 128            # dh chunks per core
NTS = SQ // 512             # 512-wide q strips
NEG = -1.0e30
SCALE = D ** -0.5

N_CORES = 8

_PROGRAM_CACHE = {}
LAST_RESULTS = None


def _chunks512(n):
    out = []
    o = 0
    while o < n:
        w = min(512, n - o)
        out.append((o, w))
        o += w
    return out


def build_program(skv):
    """Build + compile the single-core SPMD Bass program for padded KV
    length `skv` (multiple of 128)."""
    if skv in _PROGRAM_CACHE:
        return _PROGRAM_CACHE[skv]

    nkv = skv // 128
    dt = mybir.dt

    nc = bacc.Bacc(
        "TRN2",
        target_bir_lowering=False,
        debug=False,
        enable_asserts=False,
        num_devices=N_CORES,
    )

    # DRAM I/O (per-core shapes)
    qT = nc.dram_tensor("qT", [E, SQ], dt.bfloat16, kind="ExternalInput").ap()
    kT = nc.dram_tensor("kT", [E, skv], dt.bfloat16, kind="ExternalInput").ap()
    vT = nc.dram_tensor("vT", [E, skv], dt.bfloat16, kind="ExternalInput").ap()
    wqT = nc.dram_tensor("wqT", [E, DHC], dt.bfloat16, kind="ExternalInput").ap()
    wkT = nc.dram_tensor("wkT", [E, DHC], dt.bfloat16, kind="ExternalInput").ap()
    wvT = nc.dram_tensor("wvT", [E, DHC], dt.bfloat16, kind="ExternalInput").ap()
    woT = nc.dram_tensor("woT", [DHC, E], dt.bfloat16, kind="ExternalInput").ap()
    mb = nc.dram_tensor("mb", [128, nkv], dt.float32, kind="ExternalInput").ap()
    outT = nc.dram_tensor("outT", [E, SQ], dt.float32, kind="ExternalOutput").ap()

    ts = bass.ts
    kvchunks = _chunks512(skv)

    with tile.TileContext(nc) as tc:
        with tc.tile_pool(name="persist", bufs=1) as pp:
            # Persistent SBUF tensors
            wv_sb = [pp.tile([128, DHC], dt.bfloat16, name=f"wv{e}", tag=f"wv{e}") for e in range(NE)]
            qh_sb = [pp.tile([128, SQ], dt.bfloat16, name=f"qh{c}", tag=f"qh{c}") for c in range(NDH)]
            kh_sb = [pp.tile([128, skv], dt.bfloat16, name=f"kh{c}", tag=f"kh{c}") for c in range(NDH)]
            # V augmented per head to [kv, 64 V | 64 ones]: the ones block
            # replicates the softmax denominator Z into PSUM rows 64-127.
            va_sb = [pp.tile([128, HPC * 2 * D], dt.bfloat16, name=f"va{j}", tag=f"va{j}") for j in range(nkv)]
            aall_sb = [pp.tile([128, SQ], dt.bfloat16, name=f"aall{c}", tag=f"aall{c}") for c in range(NDH)]
            mb_sb = pp.tile([128, nkv], dt.float32, name="mbt", tag="mbt")

            # ones half-blocks of the augmented V (bf16 1.0)
            for j in range(nkv):
                va3 = va_sb[j].rearrange("p (h x) -> p h x", x=2 * D)
                nc.gpsimd.memset(va3[:, :, D:2 * D], 1.0)

            wo_sb = [pp.tile([128, E], dt.bfloat16, name=f"wo{c}", tag=f"wo{c}") for c in range(NDH)]

            vvp = tc.alloc_tile_pool(name="vvp", bufs=1)
            v_sb = [vvp.tile([128, skv], dt.bfloat16, name=f"v{e}", tag=f"v{e}") for e in range(NE)]
            vip = tc.alloc_tile_pool(name="vinp", bufs=1)
            q_sb = [vip.tile([128, SQ], dt.bfloat16, name=f"q{e}", tag=f"q{e}") for e in range(NE)]
            k_sb = [vip.tile([128, skv], dt.bfloat16, name=f"k{e}", tag=f"k{e}") for e in range(NE)]
            wq_sb = [vip.tile([128, DHC], dt.bfloat16, name=f"wq{e}", tag=f"wq{e}") for e in range(NE)]
            wk_sb = [vip.tile([128, DHC], dt.bfloat16, name=f"wk{e}", tag=f"wk{e}") for e in range(NE)]

            # DMA order matches first-use order
            for e in range(NE):
                nc.sync.dma_start(wq_sb[e][:], wqT[ts(e, 128), :])
                nc.sync.dma_start(q_sb[e][:], qT[ts(e, 128), :])
            for e in range(NE):
                nc.sync.dma_start(wk_sb[e][:], wkT[ts(e, 128), :])
                nc.sync.dma_start(k_sb[e][:], kT[ts(e, 128), :])
            nc.sync.dma_start(mb_sb[:], mb[:])
            for e in range(NE):
                nc.sync.dma_start(wv_sb[e][:], wvT[ts(e, 128), :])
                nc.sync.dma_start(v_sb[e][:], vT[ts(e, 128), :])
            for cdh in range(NDH):
                nc.sync.dma_start(wo_sb[cdh][:], woT[ts(cdh, 128), :])

            # PSUM pools, alive for the whole program
            scp = tc.alloc_tile_pool(name="scp", bufs=2, space="PSUM")
            avp = tc.alloc_tile_pool(name="avp", bufs=2, space="PSUM")
            pjp = tc.alloc_tile_pool(name="pjp", bufs=2, space="PSUM")

            # ---------------- work items ----------------
            def q_item(c, t):
                qps = pjp.tile([128, 512], dt.float32, name="pj", tag="pj")
                for e in range(NE):
                    nc.tensor.matmul(
                        qps[:], wq_sb[e][:, ts(c, 128)], q_sb[e][:, ts(t, 512)],
                        start=(e == 0), stop=(e == NE - 1),
                    )
                nc.vector.tensor_copy(qh_sb[c][:, ts(t, 512)], qps[:])

            def k_item(c, ci):
                o, w = kvchunks[ci]
                kps = pjp.tile([128, 512], dt.float32, name="pj", tag="pj")
                for e in range(NE):
                    nc.tensor.matmul(
                        kps[:, 0:w], wk_sb[e][:, ts(c, 128)], k_sb[e][:, o:o + w],
                        start=(e == 0), stop=(e == NE - 1),
                    )
                nc.vector.tensor_copy(kh_sb[c][:, o:o + w], kps[:, 0:w])

            def v_item(j):
                vps = pjp.tile([128, 512], dt.float32, name="pj", tag="pj")
                for e in range(NE):
                    nc.tensor.matmul(
                        vps[:], v_sb[e][:, ts(j, 128)], wv_sb[e][:],
                        start=(e == 0), stop=(e == NE - 1),
                    )
                dst = va_sb[j].rearrange("p (h x) -> p h x", x=2 * D)[:, :, 0:D]
                src = vps.rearrange("p (h x) -> p h x", x=D)
                nc.vector.tensor_copy(dst, src)

            # ---------------- all Q/K projections up front ----------------
            # (paired attention needs both heads' P tiles resident, so the
            # q/k input pool must release before the P pool allocates)
            for c in range(NDH):
                for t in range(NTS):
                    q_item(c, t)
                for ci in range(len(kvchunks)):
                    k_item(c, ci)
            vip.release()
            npool = tc.alloc_tile_pool(name="npool", bufs=1)

            # ---------------- head-slot pipeline ----------------
            def av_half(hp, half, p_prev):
                # AV for q-half `half` of head hp, j-major over a single
                # [128,1024] PSUM tile (one weight load per kv chunk).
                cp, rp = hp // 2, hp % 2
                a2 = avp.tile([128, 1024], dt.float32, name="a2", tag="a2", bufs=1)
                for j in range(nkv):
                    for s in range(2):
                        nc.tensor.matmul(
                            a2[:, ts(s, 512)],
                            va_sb[j][:, hp * 2 * D:(hp + 1) * 2 * D],
                            p_prev[j][:, half * 1024 + s * 512:half * 1024 + (s + 1) * 512],
                            start=(j == 0), stop=(j == nkv - 1),
                        )
                # Z replicas (PSUM rows 64-127) -> base-0 SBUF via standard
                # cross-partition copy (custom-DVE recip needs base-0
                # operands); then one DVE mul reads A straight from PSUM.
                zt = npool.tile([64, 1024], dt.float32, name="zt", tag="zt")
                nc.vector.tensor_copy(zt[:], a2[D:2 * D, :])
                rz = npool.tile([64, 1024], dt.float32, name="rz", tag="rz")
                nc.vector.reciprocal_approx_fast(rz[:], zt[:])
                nc.vector.tensor_mul(
                    aall_sb[cp][rp * 64:(rp + 1) * 64, half * 1024:(half + 1) * 1024],
                    a2[0:D, :], rz[:],
                )

            with tc.tile_pool(name="ppool", bufs=1) as ppool:
                pA_prev = pB_prev = None
                for ps in range(NDH + 1):
                    if ps < NDH:
                        qhA = qh_sb[ps][0:64, :]
                        khA = kh_sb[ps][0:64, :]
                        qhB = qh_sb[ps][64:128, :]
                        khB = kh_sb[ps][64:128, :]
                        pA, pB = [], []

                    for half in range(2):
                        if ps > 0:
                            # AV for the previous pair's even head on this
                            # q-half; the odd head's follows scores j=0 so
                            # its PSUM accumulator has drained by then.
                            av_half(2 * (ps - 1), half, pA_prev)
                        if ps < NDH:
                            for j in range(nkv):
                                if ps >= 1:
                                    for f in range(2):
                                        nc.tensor.ldweights(weights=kh_sb[0][:, ts(f, 128)])
                                if half == 0:
                                    ptA = ppool.tile([128, SQ], dt.bfloat16, name=f"pa{j}", tag=f"pa{j}")
                                    ptB = ppool.tile([128, SQ], dt.bfloat16, name=f"pb{j}", tag=f"pb{j}")
                                    pA.append(ptA)
                                    pB.append(ptB)
                                # Paired scores: even head on PE rows 0-63,
                                # odd head on rows 64-127 (tile_position from
                                # the operand base partitions); interleaved
                                # matmuls run concurrently on disjoint row
                                # halves of the array.
                                scA = scp.tile([128, 1024], dt.float32, name="scA", tag="scA", bufs=1)
                                scB = scp.tile([128, 1024], dt.float32, name="scB", tag="scB", bufs=1)
                                for s in range(2):
                                    lo = half * 1024 + s * 512
                                    nc.tensor.matmul(
                                        scA[:, ts(s, 512)], khA[:, ts(j, 128)],
                                        qhA[:, lo:lo + 512], start=True, stop=True,
                                    )
                                    nc.tensor.matmul(
                                        scB[:, ts(s, 512)], khB[:, ts(j, 128)],
                                        qhB[:, lo:lo + 512], start=True, stop=True,
                                    )
                                nc.scalar.activation(
                                    pA[j][:, half * 1024:(half + 1) * 1024], scA[:],
                                    mybir.ActivationFunctionType.Exp,
                                    bias=mb_sb[:, j:j + 1], scale=SCALE,
                                )
                                nc.scalar.activation(
                                    pB[j][:, half * 1024:(half + 1) * 1024], scB[:],
                                    mybir.ActivationFunctionType.Exp,
                                    bias=mb_sb[:, j:j + 1], scale=SCALE,
                                )
                                if j == 0 and ps > 0:
                                    av_half(2 * (ps - 1) + 1, half, pB_prev)
                                if ps == 0 and j % 2 == half:
                                    v_item(j)
                        elif ps > 0:
                            av_half(2 * (ps - 1) + 1, half, pB_prev)

                    if ps < NDH:
                        pA_prev, pB_prev = pA, pB

            # ---------------- out projection ----------------
            with tc.tile_pool(name="opool", bufs=4) as opool:
                for eo in range(NE):
                    for t in range(NTS):
                        ops = pjp.tile([128, 512], dt.float32, name="pj", tag="pj")
                        for cdh in range(NDH):
                            nc.tensor.matmul(
                                ops[:], wo_sb[cdh][:, ts(eo, 128)], aall_sb[cdh][:, ts(t, 512)],
                                start=(cdh == 0), stop=(cdh == NDH - 1),
                            )
                        ob = opool.tile([128, 512], dt.float32, name="ob", tag="ob")
                        nc.vector.tensor_copy(ob[:], ops[:])
                        nc.sync.dma_start(outT[ts(eo, 128), ts(t, 512)], ob[:])

            npool.release()
            pjp.release()
            avp.release()
            scp.release()
            vvp.release()

    nc.compile()
    _PROGRAM_CACHE[skv] = nc
    return nc


def make_in_maps(q, k, v, mask, Wq, Wk, Wv, Wo, skv):
    """Host-side shard/compact/transpose/cast. Returns per-core input dicts."""
    in_maps = []
    valid = mask != 0
    for core in range(N_CORES):
        b, hg = core // 2, core % 2
        idx = np.nonzero(valid[b])[0]
        cnt = len(idx)

        kc = np.zeros((skv, E), np.float32)
        vc = np.zeros((skv, E), np.float32)
        kc[:cnt] = k[b][idx]
        vc[:cnt] = v[b][idx]

        mbias = np.zeros((skv,), np.float32)
        mbias[cnt:] = NEG
        # [128, nkv]: column j = kv chunk j
        mb2 = np.ascontiguousarray(mbias.reshape(-1, 128).T)

        rows = slice(hg * DHC, (hg + 1) * DHC)
        in_maps.append(dict(
            qT=np.ascontiguousarray(q[b].T).astype(BF16),
            kT=np.ascontiguousarray(kc.T).astype(BF16),
            vT=np.ascontiguousarray(vc.T).astype(BF16),
            wqT=np.ascontiguousarray(Wq[rows, :].T).astype(BF16),
            wkT=np.ascontiguousarray(Wk[rows, :].T).astype(BF16),
            wvT=np.ascontiguousarray(Wv[rows, :].T).astype(BF16),
            woT=np.ascontiguousarray(Wo[:, rows].T).astype(BF16),
            mb=mb2,
        ))
    return in_maps


def _numpy_fallback(q, k, v, mask, Wq, bq, Wk, bk, Wv, bv, Wo, bo):
    out = np.zeros((B, SQ, E), np.float32)
    for b in range(B):
        qh = (q[b] @ Wq.T + bq).reshape(SQ, H_TOT, D).transpose(1, 0, 2)
        kh = (k[b] @ Wk.T + bk).reshape(-1, H_TOT, D).transpose(1, 0, 2)
        vh = (v[b] @ Wv.T + bv).reshape(-1, H_TOT, D).transpose(1, 0, 2)
        att = np.einsum("hqd,hkd->hqk", qh, kh) * SCALE
        valid = mask[b] != 0
        if not valid.any():
            out[b] = bo
            continue
        att = np.where(valid[None, None, :], att, -np.inf)
        att = att - att.max(-1, keepdims=True)
        att = np.exp(att)
        att /= att.sum(-1, keepdims=True)
        o = np.einsum("hqk,hkd->hqd", att, vh)
        o = o.transpose(1, 0, 2).reshape(SQ, E)
        out[b] = o @ Wo.T + bo
    return out


def kernel(**inputs):
    global LAST_RESULTS
    q = np.asarray(inputs["q"], np.float32)
    k = np.asarray(inputs["k"], np.float32)
    v = np.asarray(inputs["v"], np.float32)
    mask = np.asarray(inputs["mask"])
    Wq, bq = np.asarray(inputs["Wq"], np.float32), np.asarray(inputs["bq"], np.float32)
    Wk, bk = np.asarray(inputs["Wk"], np.float32), np.asarray(inputs["bk"], np.float32)
    Wv, bv = np.asarray(inputs["Wv"], np.float32), np.asarray(inputs["bv"], np.float32)
    Wo, bo = np.asarray(inputs["Wo"], np.float32), np.asarray(inputs["bo"], np.float32)

    if any(np.abs(x).max() > 0 for x in (bq, bk, bv)):
        # q/k/v biases are zero in this problem's setup; a nonzero bias
        # would need the augmented-contraction path, so fall back.
        return _numpy_fallback(q, k, v, mask, Wq, bq, Wk, bk, Wv, bv, Wo, bo)

    valid = mask != 0
    counts = valid.sum(axis=1)
    if counts.max() == 0:
        return np.broadcast_to(bo, (B, SQ, E)).astype(np.float32).copy()

    skv = int(-(-counts.max() // 128) * 128)
    nc = build_program(skv)
    in_maps = make_in_maps(q, k, v, mask, Wq, Wk, Wv, Wo, skv)

    res = bass_utils.run_bass_kernel_spmd(nc, in_maps, core_ids=list(range(N_CORES)))
    LAST_RESULTS = res

    out = np.empty((B, SQ, E), np.float32)
    for b in range(B):
        if counts[b] == 0:
            out[b] = bo
        else:
            p0 = res.results[2 * b]["outT"]
            p1 = res.results[2 * b + 1]["outT"]
            out[b] = p0.T + p1.T + bo
    return out


# revision 22
# speedup vs baseline: 1.2097x; 1.2097x over previous
"""Multi-head attention (batched, key-padding mask) Trainium2 Bass kernel.

Problem: nn_MultiHeadBatched
  q,k,v: [B=4, S=2048, E=1024] fp32; mask: [B, 2048] int32 (key padding)
  16 heads, head_dim 64; torch-Linear style q/k/v/out projections.

Sharding (8 cores): core c handles batch b=c//2 and head group hg=c%2
(8 heads each).  q/k/v projections are column-parallel over the head
group; out-projection is row-parallel — each core produces a partial
[E, Sq] output and the host sums the two partials per batch (+ bo).

v3 structure (single NeuronCore program, SPMD over 8 cores):
  - Host compacts KV per batch to the valid (mask!=0) positions, padded
    to a multiple of 128 (SKV); pad positions get an additive -1e30 exp
    bias (folded into the ScalarE activation).
  - Scores transposed ([kv, q]); softmax denominator Z from an all-ones
    65th column on each head's V (row 64 of the AV accumulation).
  - Head-slot pipeline with HALF-phases: slot h runs
      AV(h-1, strips 0-1) ; scores+exp(h, q-half 0) ;
      AV(h-1, strips 2-3) ; scores+exp(h, q-half 1)
    so the single-buffered P tiles ([128,2048] per kv chunk) free in
    halves just before exp needs them, and ScalarE stays busy across the
    slot boundary (previous half's exps overlap this slot's AV).
  - Q/K projections for head-pairs 1-3 are background items woven into
    the j-loops, so the exp stream starts ~15us into the kernel.
  - AV runs j-major per q-half into one [128,1024] PSUM tile (2 banks);
    V is augmented per head to [64 V | 64 ones] columns so the AV output
    carries Z replicated across rows 64-127, partition-aligned with A.
    Each half is normalized immediately: Z rows -> base-0 SBUF (standard
    cross-partition copy), custom-DVE reciprocal_approx_fast (base-0
    operands only — ISA lowering drops base_partition), then one DVE mul
    reading A straight from PSUM into aall (bf16).
  - PSUM: scores 2x[128,1024] (4) + AV [128,1024] (2) + proj 2x[128,512]
    (2) = 8 banks.
"""

import os
import sys

import numpy as np

sys.path.insert(0, "/opt/trn_rl_repo")

import concourse.bass as bass
import concourse.bacc as bacc
import concourse.mybir as mybir
import concourse.tile as tile
from concourse import bass_utils

import ml_dtypes

BF16 = ml_dtypes.bfloat16

B, SQ, E = 4, 2048, 1024
H_TOT, D = 16, 64
HPC = H_TOT // 2            # heads per core (head-group split in 2)
DHC = HPC * D               # 512 projected channels per core
NE = E // 128               # contraction chunks
NDH = DHC // 128            # dh chunks per core
NTS = SQ // 512             # 512-wide q strips
NEG = -1.0e30
SCALE = D ** -0.5

N_CORES = 8

_PROGRAM_CACHE = {}
LAST_RESULTS = None


def _chunks512(n):
    out = []
    o = 0
    while o < n:
        w = min(512, n - o)
        out.append((o, w))
        o += w
    return out


def build_program(skv):
    """Build + compile the single-core SPMD Bass program for padded KV
    length `skv` (multiple of 128)."""
    if skv in _PROGRAM_CACHE:
        return _PROGRAM_CACHE[skv]

    nkv = skv // 128
    dt = mybir.dt

    nc = bacc.Bacc(
        "TRN2",
        target_bir_lowering=False,
        debug=False,
        enable_asserts=False,
        num_devices=N_CORES,
    )

    # DRAM I/O (per-core shapes)
    qT = nc.dram_tensor("qT", [E, SQ], dt.bfloat16, kind="ExternalInput").ap()
    kT = nc.dram_tensor("kT", [E, skv], dt.bfloat16, kind="ExternalInput").ap()
    vT = nc.dram_tensor("vT", [E, skv], dt.bfloat16, kind="ExternalInput").ap()
    wqT = nc.dram_tensor("wqT", [E, DHC], dt.bfloat16, kind="ExternalInput").ap()
    wkT = nc.dram_tensor("wkT", [E, DHC], dt.bfloat16, kind="ExternalInput").ap()
    wvT = nc.dram_tensor("wvT", [E, DHC], dt.bfloat16, kind="ExternalInput").ap()
    woT = nc.dram_tensor("woT", [DHC, E], dt.bfloat16, kind="ExternalInput").ap()
    mb = nc.dram_tensor("mb", [128, nkv], dt.float32, kind="ExternalInput").ap()
    outT = nc.dram_tensor("outT", [E, SQ], dt.bfloat16, kind="ExternalOutput").ap()

    ts = bass.ts
    kvchunks = _chunks512(skv)

    with tile.TileContext(nc) as tc:
        with tc.tile_pool(name="persist", bufs=1) as pp:
            # Persistent SBUF tensors
            wq_sb = [pp.tile([128, DHC], dt.bfloat16, name=f"wq{e}", tag=f"wq{e}") for e in range(NE)]
            wk_sb = [pp.tile([128, DHC], dt.bfloat16, name=f"wk{e}", tag=f"wk{e}") for e in range(NE)]
            wv_sb = [pp.tile([128, DHC], dt.bfloat16, name=f"wv{e}", tag=f"wv{e}") for e in range(NE)]
            qh_sb = [pp.tile([128, SQ], dt.bfloat16, name=f"qh{c}", tag=f"qh{c}") for c in range(NDH)]
            kh_sb = [pp.tile([128, skv], dt.bfloat16, name=f"kh{c}", tag=f"kh{c}") for c in range(NDH)]
            # V augmented per head to [kv, 64 V | 64 ones]: the ones block
            # replicates the softmax denominator Z into PSUM rows 64-127.
            va_sb = [pp.tile([128, HPC * 2 * D], dt.bfloat16, name=f"va{j}", tag=f"va{j}") for j in range(nkv)]
            aall_sb = [pp.tile([128, SQ], dt.bfloat16, name=f"aall{c}", tag=f"aall{c}") for c in range(NDH)]
            mb_sb = pp.tile([128, nkv], dt.float32, name="mbt", tag="mbt")

            # ones half-blocks of the augmented V (bf16 1.0)
            for j in range(nkv):
                va3 = va_sb[j].rearrange("p (h x) -> p h x", x=2 * D)
                nc.gpsimd.memset(va3[:, :, D:2 * D], 1.0)

            # Startup: ~4us of dependency-free weight loads on the memset
            # ones block warm the HAM clock gate during the DMA-bound ramp,
            # and a dummy exp preloads the ACT table set.
            warm_sb = pp.tile([128, D], dt.bfloat16, name="warm", tag="warm")
            ones0 = va_sb[0].rearrange("p (h x) -> p h x", x=2 * D)[:, 0, D:2 * D]
            for _ in range(72):
                nc.tensor.ldweights(weights=ones0)
            nc.scalar.activation(warm_sb[:], ones0,
                                 mybir.ActivationFunctionType.Exp)

            vip = tc.alloc_tile_pool(name="vinp", bufs=1)
            q_sb = [vip.tile([128, SQ], dt.bfloat16, name=f"q{e}", tag=f"q{e}") for e in range(NE)]
            k_sb = [vip.tile([128, skv], dt.bfloat16, name=f"k{e}", tag=f"k{e}") for e in range(NE)]
            v_sb = [vip.tile([128, skv], dt.bfloat16, name=f"v{e}", tag=f"v{e}") for e in range(NE)]

            # DMA order matches first-use order
            for e in range(NE):
                nc.sync.dma_start(wq_sb[e][:], wqT[ts(e, 128), :])
                nc.sync.dma_start(q_sb[e][:], qT[ts(e, 128), :])
            for e in range(NE):
                nc.sync.dma_start(wk_sb[e][:], wkT[ts(e, 128), :])
                nc.sync.dma_start(k_sb[e][:], kT[ts(e, 128), :])
            nc.sync.dma_start(mb_sb[:], mb[:])
            for e in range(NE):
                nc.sync.dma_start(wv_sb[e][:], wvT[ts(e, 128), :])
                nc.sync.dma_start(v_sb[e][:], vT[ts(e, 128), :])

            # PSUM pools, alive for the whole program
            scp = tc.alloc_tile_pool(name="scp", bufs=2, space="PSUM")
            avp = tc.alloc_tile_pool(name="avp", bufs=2, space="PSUM")
            pjp = tc.alloc_tile_pool(name="pjp", bufs=2, space="PSUM")
            npool = tc.alloc_tile_pool(name="npool", bufs=2)

            # ---------------- work items ----------------
            def q_item(c, t):
                qps = pjp.tile([128, 512], dt.float32, name="pj", tag="pj")
                for e in range(NE):
                    nc.tensor.matmul(
                        qps[:], wq_sb[e][:, ts(c, 128)], q_sb[e][:, ts(t, 512)],
                        start=(e == 0), stop=(e == NE - 1),
                    )
                nc.vector.tensor_copy(qh_sb[c][:, ts(t, 512)], qps[:])

            def k_item(c, ci):
                o, w = kvchunks[ci]
                kps = pjp.tile([128, 512], dt.float32, name="pj", tag="pj")
                for e in range(NE):
                    nc.tensor.matmul(
                        kps[:, 0:w], wk_sb[e][:, ts(c, 128)], k_sb[e][:, o:o + w],
                        start=(e == 0), stop=(e == NE - 1),
                    )
                nc.vector.tensor_copy(kh_sb[c][:, o:o + w], kps[:, 0:w])

            def v_item(j):
                vps = pjp.tile([128, 512], dt.float32, name="pj", tag="pj")
                for e in range(NE):
                    nc.tensor.matmul(
                        vps[:], v_sb[e][:, ts(j, 128)], wv_sb[e][:],
                        start=(e == 0), stop=(e == NE - 1),
                    )
                dst = va_sb[j].rearrange("p (h x) -> p h x", x=2 * D)[:, :, 0:D]
                src = vps.rearrange("p (h x) -> p h x", x=D)
                nc.vector.tensor_copy(dst, src)

            # Background queue: Q/K projections for pairs 1-3, popped inside
            # the slot j-loops.  Pair c is fully drained well before slot 2c.
            bg = []
            for c in range(1, NDH):
                for t in range(NTS):
                    bg.append((q_item, c, t))
                for ci in range(len(kvchunks)):
                    bg.append((k_item, c, ci))
            bg.reverse()  # pop() from the end
            nitems = len(bg)
            # cumulative items to drain by end of slot h (pair c by slot 2c-1)
            per_pair = nitems // 3
            bg_deadline = {0: 4, 1: per_pair, 2: per_pair + 4, 3: 2 * per_pair,
                           4: 2 * per_pair + 4, 5: nitems}
            bg_done = 0

            # ---------------- prologue: pair-0 projections ----------------
            for t in range(NTS):
                q_item(0, t)
            for ci in range(len(kvchunks)):
                k_item(0, ci)

            # ---------------- head-slot pipeline ----------------
            def av_half(hp, half, p_prev):
                # AV for q-half `half` of head hp, j-major over a single
                # [128,1024] PSUM tile (one weight load per kv chunk).
                cp, rp = hp // 2, hp % 2
                a2 = avp.tile([128, 1024], dt.float32, name="a2", tag="a2", bufs=1)
                for j in range(nkv):
                    for s in range(2):
                        nc.tensor.matmul(
                            a2[:, ts(s, 512)],
                            va_sb[j][:, hp * 2 * D:(hp + 1) * 2 * D],
                            p_prev[j][:, half * 1024 + s * 512:half * 1024 + (s + 1) * 512],
                            start=(j == 0), stop=(j == nkv - 1),
                        )
                # Z replicas (PSUM rows 64-127) -> base-0 SBUF via standard
                # cross-partition copy (custom-DVE recip needs base-0
                # operands); then one DVE mul reads A straight from PSUM.
                zt = npool.tile([64, 1024], dt.float32, name="zt", tag="zt")
                nc.vector.tensor_copy(zt[:], a2[D:2 * D, :])
                rz = npool.tile([64, 1024], dt.float32, name="rz", tag="rz")
                nc.vector.reciprocal_approx_fast(rz[:], zt[:])
                nc.vector.tensor_mul(
                    aall_sb[cp][rp * 64:(rp + 1) * 64, half * 1024:(half + 1) * 1024],
                    a2[0:D, :], rz[:],
                )

            with tc.tile_pool(name="ppool", bufs=1) as ppool:
                p_prev = None
                for h in range(HPC + 1):
                    if h < HPC:
                        c, r = h // 2, h % 2
                        qh_h = qh_sb[c][r * 64:(r + 1) * 64, :]
                        kh_h = kh_sb[c][r * 64:(r + 1) * 64, :]
                        p_cur = []
                    target = bg_deadline.get(h, nitems)

                    for half in range(2):
                        if h > 0:
                            # AV for the previous head covering this q-half;
                            # frees the P columns exp below rewrites.
                            av_half(h - 1, half, p_prev)
                        if h < HPC:
                            for j in range(nkv):
                                if h >= 5:
                                    nc.tensor.ldweights(weights=wq_sb[0][:, 0:128])
                                if half == 0:
                                    pt = ppool.tile([128, SQ], dt.bfloat16, name=f"p{j}", tag=f"p{j}")
                                    p_cur.append(pt)
                                sc = scp.tile([128, 1024], dt.float32, name="sc", tag="sc")
                                for s in range(2):
                                    nc.tensor.matmul(
                                        sc[:, ts(s, 512)],
                                        kh_h[:, ts(j, 128)],
                                        qh_h[:, half * 1024 + s * 512:half * 1024 + (s + 1) * 512],
                                        start=True, stop=True,
                                    )
                                nc.scalar.activation(
                                    p_cur[j][:, half * 1024:(half + 1) * 1024], sc[:],
                                    mybir.ActivationFunctionType.Exp,
                                    bias=mb_sb[:, j:j + 1], scale=SCALE,
                                )
                                if h == 0 and j % 2 == half:
                                    v_item(j)
                                # pace the background projections
                                want = ((2 * nkv) * target) // (2 * nkv)  # simple full-slot target
                                want = ((half * nkv + j + 1) * target + 2 * nkv - 1) // (2 * nkv)
                                while bg_done < want and bg:
                                    fn, a, b_ = bg.pop()
                                    fn(a, b_)
                                    bg_done += 1

                    p_prev = p_cur if h < HPC else None

            # ---------------- out projection ----------------
            with (
                tc.tile_pool(name="wop", bufs=1) as wop,
                tc.tile_pool(name="opool", bufs=4) as opool,
            ):
                wo_sb = [wop.tile([128, E], dt.bfloat16, name=f"wo{c}", tag=f"wo{c}") for c in range(NDH)]
                for cdh in range(NDH):
                    nc.sync.dma_start(wo_sb[cdh][:], woT[ts(cdh, 128), :])
                for eo in range(NE):
                    for t in range(NTS):
                        ops = pjp.tile([128, 512], dt.float32, name="pj", tag="pj")
                        for cdh in range(NDH):
                            nc.tensor.matmul(
                                ops[:], wo_sb[cdh][:, ts(eo, 128)], aall_sb[cdh][:, ts(t, 512)],
                                start=(cdh == 0), stop=(cdh == NDH - 1),
                            )
                        ob = opool.tile([128, 512], dt.bfloat16, name="ob", tag="ob")
                        nc.vector.tensor_copy(ob[:], ops[:])
                        nc.sync.dma_start(outT[ts(eo, 128), ts(t, 512)], ob[:])

            npool.release()
            pjp.release()
            avp.release()
            scp.release()
            vip.release()

    nc.compile()
    _PROGRAM_CACHE[skv] = nc
    return nc


def make_in_maps(q, k, v, mask, Wq, Wk, Wv, Wo, skv):
    """Host-side shard/compact/transpose/cast. Returns per-core input dicts."""
    in_maps = []
    valid = mask != 0
    for core in range(N_CORES):
        b, hg = core // 2, core % 2
        idx = np.nonzero(valid[b])[0]
        cnt = len(idx)

        kc = np.zeros((skv, E), np.float32)
        vc = np.zeros((skv, E), np.float32)
        kc[:cnt] = k[b][idx]
        vc[:cnt] = v[b][idx]

        mbias = np.zeros((skv,), np.float32)
        mbias[cnt:] = NEG
        # [128, nkv]: column j = kv chunk j
        mb2 = np.ascontiguousarray(mbias.reshape(-1, 128).T)

        rows = slice(hg * DHC, (hg + 1) * DHC)
        in_maps.append(dict(
            qT=np.ascontiguousarray(q[b].T).astype(BF16),
            kT=np.ascontiguousarray(kc.T).astype(BF16),
            vT=np.ascontiguousarray(vc.T).astype(BF16),
            wqT=np.ascontiguousarray(Wq[rows, :].T).astype(BF16),
            wkT=np.ascontiguousarray(Wk[rows, :].T).astype(BF16),
            wvT=np.ascontiguousarray(Wv[rows, :].T).astype(BF16),
            woT=np.ascontiguousarray(Wo[:, rows].T).astype(BF16),
            mb=mb2,
        ))
    return in_maps


def _numpy_fallback(q, k, v, mask, Wq, bq, Wk, bk, Wv, bv, Wo, bo):
    out = np.zeros((B, SQ, E), np.float32)
    for b in range(B):
        qh = (q[b] @ Wq.T + bq).reshape(SQ, H_TOT, D).transpose(1, 0, 2)
        kh = (k[b] @ Wk.T + bk).reshape(-1, H_TOT, D).transpose(1, 0, 2)
        vh = (v[b] @ Wv.T + bv).reshape(-1, H_TOT, D).transpose(1, 0, 2)
        att = np.einsum("hqd,hkd->hqk", qh, kh) * SCALE
        valid = mask[b] != 0
        if not valid.any():
            out[b] = bo
            continue
        att = np.where(valid[None, None, :], att, -np.inf)
        att = att - att.max(-1, keepdims=True)
        att = np.exp(att)
        att /= att.sum(-1, keepdims=True)
        o = np.einsum("hqk,hkd->hqd", att, vh)
        o = o.transpose(1, 0, 2).reshape(SQ, E)
        out[b] = o @ Wo.T + bo
    return out


def kernel(**inputs):
    global LAST_RESULTS
    q = np.asarray(inputs["q"], np.float32)
    k = np.asarray(inputs["k"], np.float32)
    v = np.asarray(inputs["v"], np.float32)
    mask = np.asarray(inputs["mask"])
    Wq, bq = np.asarray(inputs["Wq"], np.float32), np.asarray(inputs["bq"], np.float32)
    Wk, bk = np.asarray(inputs["Wk"], np.float32), np.asarray(inputs["bk"], np.float32)
    Wv, bv = np.asarray(inputs["Wv"], np.float32), np.asarray(inputs["bv"], np.float32)
    Wo, bo = np.asarray(inputs["Wo"], np.float32), np.asarray(inputs["bo"], np.float32)

    if any(np.abs(x).max() > 0 for x in (bq, bk, bv)):
        # q/k/v biases are zero in this problem's setup; a nonzero bias
        # would need the augmented-contraction path, so fall back.
        return _numpy_fallback(q, k, v, mask, Wq, bq, Wk, bk, Wv, bv, Wo, bo)

    valid = mask != 0
    counts = valid.sum(axis=1)
    if counts.max() == 0:
        return np.broadcast_to(bo, (B, SQ, E)).astype(np.float32).copy()

    skv = int(-(-counts.max() // 128) * 128)
    nc = build_program(skv)
    in_maps = make_in_maps(q, k, v, mask, Wq, Wk, Wv, Wo, skv)

    res = bass_utils.run_bass_kernel_spmd(nc, in_maps, core_ids=list(range(N_CORES)))
    LAST_RESULTS = res

    out = np.empty((B, SQ, E), np.float32)
    for b in range(B):
        if counts[b] == 0:
            out[b] = bo
        else:
            p0 = np.asarray(res.results[2 * b]["outT"], np.float32)
            p1 = np.asarray(res.results[2 * b + 1]["outT"], np.float32)
            out[b] = p0.T + p1.T + bo
    return out
